# revision 4
# baseline (speedup 1.0000x reference)
"""AttnBlock (GroupNorm + single-head LxL attention + residual) on 8 trn2 cores.

Data-parallel over batch: core b handles sample b (full 2048x2048 attention).

fp8 version: all large matmuls (QKV convs, W~=WoV, S=K^T Q, Out=P~W~, row
sums) run as float8e4 (e4m3) with perf_mode=DoubleRow: the PE virtualizes to
128x256 (2 fp8 weights/cell), so each matmul contracts K=256 per instruction
at ~2x the f32r FLOP rate. Operands live in "pair layout": channel c maps to
(ki=c%128, ko=(c//128)%2, t=c//256), SBUF tiles [128, 2, X] so the 3D AP
[Ki, Ko=2, free] matches the DoubleRow interleave (ko stride is a multiple
of 16 B).

softmax: P~ = exp(S/sqrt(C) - 2) stored fp8 (max exp(6.9-2) ~ 131 < 240
e4m3 max); the -2 offset cancels in the normalization. Row sums accumulate
as fp8 ones-matmuls over the same quantized P~, so numerator/denominator
are consistent.

x, xt (residual, x^T + bo) stream in bf16; output yt is written bf16 and
upcast on host. Measured numerics vs fp32 reference: rms rel err ~6e-3
(gate 2e-2).

Layout per core:
  x            : (C, L) bf16, 4 tiles [128, 2048] (stats + GN input)
  H8,Q8,K8,V8  : fp8 pair tiles, 2 x [128, 2, 2048] each
  S^T          : [j, i] PSUM tiles -> exp -> P~^T fp8 pair tiles [128,2,512]
  W~^T = (WoV)^T: 8 fp8 pair tiles [128, 2, 512] (j pairs x channels)
  out^T        : PSUM [128, 512] per i-slice; evac fuses *1/rowsum + x^T
                 (scalar_tensor_tensor) into bf16 yt tiles.
"""

import numpy as np
import ml_dtypes

C = 512
L = 2048
G = 32
GS = C // G          # 16 channels per group
EPS = 1e-6
CT = C // 128        # 4 channel tiles
T = 2                # 256-channel DoubleRow k-tiles
JT = L // 128        # 16 j tiles
JP = JT // 2         # 8 j pair tiles
NB = 512             # matmul moving free dim / i-block size
LB = L // NB         # 4 i-blocks
NCORES = 8
EXP_BIAS = -2.0
NWARM = 42

_CACHE = {}


def _build():
    import concourse.bacc as bacc
    import concourse.tile as tile
    from concourse import mybir
    from concourse.alu_op_type import AluOpType
    from contextlib import ExitStack

    F32 = mybir.dt.float32
    BF16 = mybir.dt.bfloat16
    F8 = mybir.dt.float8e4
    DR = mybir.MatmulPerfMode.DoubleRow
    AF = mybir.ActivationFunctionType
    AX = mybir.AxisListType

    nc = bacc.Bacc("TRN2", target_bir_lowering=False, debug=False, num_devices=NCORES)

    _ctr = [0]

    def nm(base):
        _ctr[0] += 1
        return f"{base}_{_ctr[0]}"

    x_d = nc.declare_dram_parameter("x", [C, L], BF16, isOutput=False)
    xt_d = nc.declare_dram_parameter("xt", [L, C], BF16, isOutput=False)
    wq8_d = nc.declare_dram_parameter("wq8", [128, T * 2 * C], F8, isOutput=False)
    wk8_d = nc.declare_dram_parameter("wk8", [128, T * 2 * C], F8, isOutput=False)
    wv8_d = nc.declare_dram_parameter("wv8", [128, T * 2 * C], F8, isOutput=False)
    wo8_d = nc.declare_dram_parameter("wo8", [128, T * 2 * C], F8, isOutput=False)
    cvec_d = nc.declare_dram_parameter("cvec", [128, 5 * CT], F32, isOutput=False)
    gmil_d = nc.declare_dram_parameter("gmil", [128, G * CT], F32, isOutput=False)
    gmT_d = nc.declare_dram_parameter("gmT", [G, C], F32, isOutput=False)
    one8_d = nc.declare_dram_parameter("one8", [128, 32], F8, isOutput=False)
    yt_d = nc.declare_dram_parameter("yt", [L, C], BF16, isOutput=True)

    scale = float(1.0 / np.sqrt(C))

    with tile.TileContext(nc) as tc, ExitStack() as ctx:
        consts = ctx.enter_context(tc.tile_pool(name="consts", bufs=1))
        small = ctx.enter_context(tc.tile_pool(name="small", bufs=4))
        x_p = ctx.enter_context(tc.tile_pool(name="xp", bufs=4))
        scr_p = ctx.enter_context(tc.tile_pool(name="scr", bufs=2))
        h8_p = ctx.enter_context(tc.tile_pool(name="h8", bufs=2))
        q8_p = ctx.enter_context(tc.tile_pool(name="q8", bufs=2))
        k8_p = ctx.enter_context(tc.tile_pool(name="k8", bufs=2))
        v8_p = ctx.enter_context(tc.tile_pool(name="v8", bufs=2))
        w8_p = ctx.enter_context(tc.tile_pool(name="w8", bufs=4))
        wt8_p = ctx.enter_context(tc.tile_pool(name="wt8", bufs=8))
        pt_p = ctx.enter_context(tc.tile_pool(name="pt", bufs=4))
        xt_p = ctx.enter_context(tc.tile_pool(name="xtp", bufs=5))
        io_p = ctx.enter_context(tc.tile_pool(name="io", bufs=4))
        ps_mm = ctx.enter_context(tc.tile_pool(name="psmm", bufs=5, space="PSUM"))
        ps_s = ctx.enter_context(tc.tile_pool(name="pss", bufs=3, space="PSUM"))

        gmil_sb = consts.tile([128, G * CT], F32, name=nm("gmil"), tag="gmil")
        nc.sync.dma_start(out=gmil_sb[:], in_=gmil_d[:, :])
        # x tiles next on the queue: stats (and everything after) gate on the
        # full x arrival, so it precedes the other loads
        x_t = []
        for ct in range(CT):
            xin = x_p.tile([128, L], BF16, name=nm("x"), tag="x")
            x_t.append(xin)
            nc.sync.dma_start(out=xin[:], in_=x_d[ct * 128:(ct + 1) * 128, :])
        ones8 = consts.tile([128, 2, 16], F8, name=nm("ones8"), tag="ones8")
        for ko in range(2):
            nc.sync.dma_start(out=ones8[:, ko, :], in_=one8_d[:, ko * 16:(ko + 1) * 16])
        cv_sb = consts.tile([128, 5 * CT], F32, name=nm("cv"), tag="cv")
        nc.sync.dma_start(out=cv_sb[:], in_=cvec_d[:, :])
        gmT_sb = consts.tile([G, C], F32, name=nm("gmT"), tag="gmT")
        nc.sync.dma_start(out=gmT_sb[:], in_=gmT_d[:, :])

        onesf = consts.tile([1, 1], F32, name=nm("onesf"), tag="onesf")
        nc.vector.memset(onesf[:], 1.0)
        eps_t = consts.tile([G, 1], F32, name=nm("eps"), tag="eps")
        nc.vector.memset(eps_t[:], EPS)
        ebias_t = consts.tile([128, 1], F32, name=nm("ebias"), tag="ebias")
        nc.vector.memset(ebias_t[:], EXP_BIAS)

        # warm-up matmuls on the early-arriving mask tile (f32): keep the PE
        # at full clock and busy while x streams in and stats run
        for i in range(NWARM):
            wps = ps_mm.tile([128, 128], F32, name=nm("warm"), tag="mm")
            nc.tensor.matmul(wps[:], gmil_sb[:, 0:128], gmil_sb[:, 0:128],
                             start=True, stop=True)

        # fp8 weights in pair layout, 2 tiles [128, 2, C] per weight
        def load_w(w_dram):
            wsb = []
            for t in range(T):
                w = w8_p.tile([128, 2, C], F8, name=nm("w"), tag="w")
                for ko in range(2):
                    off = t * 2 * C + ko * C
                    nc.sync.dma_start(out=w[:, ko, :], in_=w_dram[:, off:off + C])
                wsb.append(w)
            return wsb

        gm_sb = [gmil_sb[:, ct * G:(ct + 1) * G] for ct in range(CT)]
        bq_t = [cv_sb[:, ct * 5 + 0:ct * 5 + 1] for ct in range(CT)]
        bk_t = [cv_sb[:, ct * 5 + 1:ct * 5 + 2] for ct in range(CT)]
        bv_t = [cv_sb[:, ct * 5 + 2:ct * 5 + 3] for ct in range(CT)]
        gnw_t = [cv_sb[:, ct * 5 + 3:ct * 5 + 4] for ct in range(CT)]
        gnb_t = [cv_sb[:, ct * 5 + 4:ct * 5 + 5] for ct in range(CT)]

        # ---- GroupNorm pass 1: per-channel sum and sum-of-squares ----
        stats = []
        for ct in range(CT):
            st = small.tile([128, 2], F32, name=nm("st"), tag=f"st{ct}")
            stats.append(st)
            xin = x_t[ct]
            nc.vector.reduce_sum(out=st[:, 0:1], in_=xin[:], axis=AX.X)
            scr = scr_p.tile([128, L], BF16, name=nm("scr"), tag="scr")
            nc.scalar.activation(out=scr[:], in_=xin[:], func=AF.Square,
                                 accum_out=st[:, 1:2])

        # group-reduce the per-channel stats: [32, 2] = sum over channels in group
        gps = ps_s.tile([G, 2], F32, name=nm("s"), tag="s")
        for ct in range(CT):
            nc.tensor.matmul(gps[:], gm_sb[ct], stats[ct][:],
                             start=(ct == 0), stop=(ct == CT - 1))
        gmv = small.tile([G, 2], F32, name=nm("gmv"), tag="gmv")
        nc.scalar.mul(out=gmv[:], in_=gps[:], mul=1.0 / (GS * L))
        msq = small.tile([G, 1], F32, name=nm("msq"), tag="msq")
        nc.vector.tensor_mul(out=msq[:], in0=gmv[:, 0:1], in1=gmv[:, 0:1])
        var = small.tile([G, 1], F32, name=nm("var"), tag="var")
        nc.vector.tensor_sub(out=var[:], in0=gmv[:, 1:2], in1=msq[:])
        rstd = small.tile([G, 1], F32, name=nm("rstd"), tag="rstd")
        nc.scalar.activation(out=rstd[:], in_=var[:], func=AF.Sqrt,
                             bias=eps_t[:], scale=1.0)
        mr = small.tile([G, 2], F32, name=nm("mr"), tag="mr")
        nc.vector.tensor_copy(out=mr[:, 0:1], in_=gmv[:, 0:1])
        nc.vector.reciprocal(out=mr[:, 1:2], in_=rstd[:])

        # broadcast group mean/rstd back to channels, fold in gn weight/bias
        s_t, t_t = [], []
        for ct in range(CT):
            bps = ps_s.tile([128, 2], F32, name=nm("s"), tag="s")
            nc.tensor.matmul(bps[:], gmT_sb[:, ct * 128:(ct + 1) * 128], mr[:],
                             start=True, stop=True)
            s_ = small.tile([128, 1], F32, name=nm("sc"), tag=f"sc{ct}")
            nc.vector.tensor_mul(out=s_[:], in0=bps[:, 1:2], in1=gnw_t[ct])
            tmp = small.tile([128, 1], F32, name=nm("tmp"), tag="tmp")
            nc.vector.tensor_mul(out=tmp[:], in0=bps[:, 0:1], in1=s_[:])
            t_ = small.tile([128, 1], F32, name=nm("tc"), tag=f"tc{ct}")
            nc.vector.tensor_sub(out=t_[:], in0=gnb_t[ct], in1=tmp[:])
            s_t.append(s_)
            t_t.append(t_)

        wq8_sb = load_w(wq8_d)

        # ---- GroupNorm pass 2: H8 = fp8(s*x + t) in pair layout,
        # chunked in conv consumption order and split across DVE/ACT ----
        h8_t = [h8_p.tile([128, 2, L], F8, name=nm("h8"), tag="h8")
                for _ in range(T)]
        for lc in range(2):
            sl = slice(lc * 1024, (lc + 1) * 1024)
            for ct in range(CT):
                dst = h8_t[ct // 2][:, ct % 2, sl]
                if ct % 2 == 0:
                    nc.vector.tensor_scalar(out=dst, in0=x_t[ct][:, sl],
                                            scalar1=s_t[ct][:],
                                            scalar2=t_t[ct][:],
                                            op0=AluOpType.mult,
                                            op1=AluOpType.add)
                else:
                    nc.scalar.activation(out=dst, in_=x_t[ct][:, sl],
                                         func=AF.Identity, bias=t_t[ct][:],
                                         scale=s_t[ct][:])

        # ---- 1x1 convs in fp8 DoubleRow; outputs in fp8 pair layout ----
        def conv(bias_t, pool, tag, wsb, act_evac=False):
            out8 = [pool.tile([128, 2, L], F8, name=nm(tag), tag=tag)
                    for _ in range(T)]
            for co in range(CT):
                pss = [ps_mm.tile([128, NB], F32, name=nm("mm"), tag="mm")
                       for _ in range(L // NB)]
                for lc in range(L // NB):
                    for t in range(T):
                        nc.tensor.matmul(
                            pss[lc][:],
                            wsb[t][:, :, co * 128:(co + 1) * 128],
                            h8_t[t][:, :, lc * NB:(lc + 1) * NB],
                            start=(t == 0), stop=(t == T - 1),
                            perf_mode=DR)
                for lc in range(L // NB):
                    dst = out8[co // 2][:, co % 2, lc * NB:(lc + 1) * NB]
                    if act_evac:
                        nc.scalar.activation(out=dst, in_=pss[lc][:],
                                             func=AF.Identity,
                                             bias=bias_t[co], scale=1.0)
                    else:
                        nc.vector.tensor_scalar_add(
                            out=dst, in0=pss[lc][:], scalar1=bias_t[co])
            return out8

        q8_t = conv(bq_t, q8_p, "q", wq8_sb)
        wk8_sb = load_w(wk8_d)
        k8_t = conv(bk_t, k8_p, "k", wk8_sb, act_evac=True)
        wv8_sb = load_w(wv8_d)
        v8_t = conv(bv_t, v8_p, "v", wv8_sb)
        wo8_sb = load_w(wo8_d)

        # ---- W~^T = (Wo V)^T, fp8 pair tiles over j ----
        wt8 = [wt8_p.tile([128, 2, C], F8, name=nm("wt"), tag="wt")
               for _ in range(JP)]
        for jt in range(JT):
            ps = ps_mm.tile([128, C], F32, name=nm("mm"), tag="mm")
            for t in range(T):
                nc.tensor.matmul(
                    ps[:],
                    v8_t[t][:, :, jt * 128:(jt + 1) * 128],
                    wo8_sb[t][:],
                    start=(t == 0), stop=(t == T - 1),
                    perf_mode=DR)
            dst = wt8[jt // 2][:, jt % 2, :]
            if jt % 2 == 0:
                nc.scalar.copy(out=dst, in_=ps[:])
            else:
                nc.vector.tensor_copy(out=dst, in_=ps[:])

        # ---- attention: blocks of 512 i columns ----
        for ib in range(LB):
            rsps = ps_s.tile([1, NB], F32, name=nm("rs"), tag="s")
            ops = [ps_mm.tile([128, C], F32, name=nm("mm"), tag="mm")
                   for _ in range(4)]
            xt_sbs = []
            for s in range(4):
                row = ib * NB + s * 128
                xt_sb = xt_p.tile([128, C], BF16, name=nm("xt"), tag="xt")
                nc.sync.dma_start(out=xt_sb[:], in_=xt_d[row:row + 128, :])
                xt_sbs.append(xt_sb)
            pt = None
            for jt in range(JT):
                jp, jo = jt // 2, jt % 2
                if jo == 0:
                    pt = pt_p.tile([128, 2, NB], F8, name=nm("p"), tag="p")
                sps = ps_s.tile([128, NB], F32, name=nm("s"), tag="s")
                for t in range(T):
                    nc.tensor.matmul(
                        sps[:],
                        k8_t[t][:, :, jt * 128:(jt + 1) * 128],
                        q8_t[t][:, :, ib * NB:(ib + 1) * NB],
                        start=(t == 0), stop=(t == T - 1),
                        perf_mode=DR)
                nc.scalar.activation(out=pt[:, jo, :], in_=sps[:], func=AF.Exp,
                                     scale=scale, bias=ebias_t[:])
                if jo == 1:
                    # row sums first: the block-tail normalize chain hangs off
                    # this, so it should finish before the last out matmuls
                    nc.tensor.matmul(rsps[:], ones8[:, :, 0:1], pt[:],
                                     start=(jp == 0), stop=(jp == JP - 1),
                                     perf_mode=DR)
                    for s in range(4):
                        nc.tensor.matmul(ops[s][:],
                                         pt[:, :, s * 128:(s + 1) * 128],
                                         wt8[jp][:],
                                         start=(jp == 0), stop=(jp == JP - 1),
                                         perf_mode=DR)
            rssb = small.tile([1, NB], F32, name=nm("rssb"), tag="rssb")
            nc.vector.tensor_copy(out=rssb[:], in_=rsps[:])
            rec4 = small.tile([128, 4], F32, name=nm("rec4"), tag="rec4")
            if ib < LB - 1:
                # mid-block: DMA scatter (PE is busy with the next block)
                rs4 = small.tile([128, 4], F32, name=nm("rs4"), tag="rs4")
                for s in range(4):
                    nc.sync.dma_start(out=rs4[:, s:s + 1],
                                      in_=rssb[0:1, s * 128:(s + 1) * 128])
                nc.vector.reciprocal(out=rec4[:], in_=rs4[:])
            else:
                # last block: K=1 transpose matmuls (PE idle, shortest chain)
                trp = ps_s.tile([128, 4], F32, name=nm("tr"), tag="s")
                for s in range(4):
                    nc.tensor.matmul(trp[:, s:s + 1],
                                     rssb[0:1, s * 128:(s + 1) * 128],
                                     onesf[:],
                                     start=True, stop=True)
                nc.vector.reciprocal(out=rec4[:], in_=trp[:])
            for s in range(4):
                row = ib * NB + s * 128
                yt_sb = io_p.tile([128, C], BF16, name=nm("yt"), tag="yt")
                # fused: yt = ops[s] * (1/rowsum) + (x^T + bo), bf16 out
                nc.vector.scalar_tensor_tensor(
                    out=yt_sb[:], in0=ops[s][:], scalar=rec4[:, s:s + 1],
                    in1=xt_sbs[s][:],
                    op0=AluOpType.mult, op1=AluOpType.add)
                nc.sync.dma_start(out=yt_d[row:row + 128, :], in_=yt_sb[:])

    nc.compile()
    return nc


def get_nc():
    if "nc" not in _CACHE:
        _CACHE["nc"] = _build()
    return _CACHE["nc"]


def _pair8(wT):
    # wT: [C, O] f32, c -> (t = c//256, ko = (c//128)%2, ki = c%128)
    # returns [128, T*2*O] fp8: free index = t*(2*O) + ko*O + o
    O = wT.shape[1]
    arr = wT.reshape(T, 2, 128, O).transpose(2, 0, 1, 3).reshape(128, T * 2 * O)
    return np.ascontiguousarray(arr.astype(ml_dtypes.float8_e4m3))


def make_in_maps(**inputs):
    x = np.asarray(inputs["x"], np.float32)
    bo = np.asarray(inputs["bo"], np.float32)
    gm = np.zeros((C, G), np.float32)
    gm[np.arange(C), np.arange(C) // GS] = 1.0
    shared = {
        "wq8": _pair8(np.asarray(inputs["wq"], np.float32).T),
        "wk8": _pair8(np.asarray(inputs["wk"], np.float32).T),
        "wv8": _pair8(np.asarray(inputs["wv"], np.float32).T),
        "wo8": _pair8(np.asarray(inputs["wo"], np.float32).T),
        "cvec": np.stack(
            [np.asarray(inputs[k], np.float32).reshape(CT, 128)
             for k in ("bq", "bk", "bv", "gn_w", "gn_b")],
            axis=-1).transpose(1, 0, 2).reshape(128, CT * 5).copy(),
        "gmil": gm.reshape(CT, 128, G).transpose(1, 0, 2).reshape(128, CT * G).copy(),
        "gmT": np.ascontiguousarray(gm.T),
        "one8": np.ones((128, 32), ml_dtypes.float8_e4m3),
    }
    in_maps = []
    for b in range(NCORES):
        m = dict(shared)
        m["x"] = np.ascontiguousarray(x[b]).astype(ml_dtypes.bfloat16)
        m["xt"] = np.ascontiguousarray(x[b].T + bo[None, :]).astype(ml_dtypes.bfloat16)
        in_maps.append(m)
    return in_maps


def kernel(**inputs):
    from concourse.bass_utils import run_bass_kernel_spmd

    nc = get_nc()
    in_maps = make_in_maps(**inputs)
    res = run_bass_kernel_spmd(nc, in_maps, core_ids=list(range(NCORES)))
    out = np.stack([np.asarray(res.results[b]["yt"]).astype(np.float32).T
                    for b in range(NCORES)])
    return np.ascontiguousarray(out, dtype=np.float32)


# revision 8
# speedup vs baseline: 1.3026x; 1.3026x over previous
"""AttnBlock (GroupNorm + single-head LxL attention + residual) on 8 trn2 cores.

Data-parallel over batch: core b handles sample b (full 2048x2048 attention).

fp8 version: all large matmuls (QKV convs, W~=WoV, S=K^T Q, Out=P~W~, row
sums) run as float8e4 (e4m3) with perf_mode=DoubleRow: the PE virtualizes to
128x256 (2 fp8 weights/cell), so each matmul contracts K=256 per instruction
at ~2x the f32r FLOP rate. Operands live in "pair layout": channel c maps to
(ki=c%128, ko=(c//128)%2, t=c//256), SBUF tiles [128, 2, X] so the 3D AP
[Ki, Ko=2, free] matches the DoubleRow interleave (ko stride is a multiple
of 16 B).

softmax: P~ = exp(S/sqrt(C) - 2) stored fp8 (max exp(6.9-2) ~ 131 < 240
e4m3 max); the -2 offset cancels in the normalization. Row sums accumulate
as fp8 ones-matmuls over the same quantized P~, so numerator/denominator
are consistent.

x, xt (residual, x^T + bo) stream in bf16; output yt is written bf16 and
upcast on host. Measured numerics vs fp32 reference: rms rel err ~6e-3
(gate 2e-2).

Layout per core:
  x            : (C, L) bf16, 4 tiles [128, 2048] (stats + GN input)
  H8,Q8,K8,V8  : fp8 pair tiles, 2 x [128, 2, 2048] each
  S^T          : [j, i] PSUM tiles -> exp -> P~^T fp8 pair tiles [128,2,512]
  W~^T = (WoV)^T: 8 fp8 pair tiles [128, 2, 512] (j pairs x channels)
  out^T        : PSUM [128, 512] per i-slice; evac fuses *1/rowsum + x^T
                 (scalar_tensor_tensor) into bf16 yt tiles.
"""

import numpy as np
import ml_dtypes

C = 512
L = 2048
G = 32
GS = C // G          # 16 channels per group
EPS = 1e-6
CT = C // 128        # 4 channel tiles
T = 2                # 256-channel DoubleRow k-tiles
JT = L // 128        # 16 j tiles
JP = JT // 2         # 8 j pair tiles
NB = 512             # matmul moving free dim / i-block size
LB = L // NB         # 4 i-blocks
NCORES = 8
EXP_BIAS = -2.0
NWARM = 42

_CACHE = {}


def _build():
    import concourse.bacc as bacc
    import concourse.tile as tile
    from concourse import mybir
    from concourse.alu_op_type import AluOpType
    from contextlib import ExitStack

    F32 = mybir.dt.float32
    BF16 = mybir.dt.bfloat16
    F8 = mybir.dt.float8e4
    DR = mybir.MatmulPerfMode.DoubleRow
    AF = mybir.ActivationFunctionType
    AX = mybir.AxisListType

    nc = bacc.Bacc("TRN2", target_bir_lowering=False, debug=False, num_devices=NCORES)

    _ctr = [0]

    def nm(base):
        _ctr[0] += 1
        return f"{base}_{_ctr[0]}"

    x_d = nc.declare_dram_parameter("x", [C, L], BF16, isOutput=False)
    xt_d = nc.declare_dram_parameter("xt", [L, C], BF16, isOutput=False)
    wq8_d = nc.declare_dram_parameter("wq8", [128, T * 2 * C], F8, isOutput=False)
    wk8_d = nc.declare_dram_parameter("wk8", [128, T * 2 * C], F8, isOutput=False)
    wv8_d = nc.declare_dram_parameter("wv8", [128, T * 2 * C], F8, isOutput=False)
    wo8_d = nc.declare_dram_parameter("wo8", [128, T * 2 * C], F8, isOutput=False)
    cvec_d = nc.declare_dram_parameter("cvec", [128, 5 * CT], F32, isOutput=False)
    gmil_d = nc.declare_dram_parameter("gmil", [128, G * CT], F32, isOutput=False)
    gmT_d = nc.declare_dram_parameter("gmT", [G, C], F32, isOutput=False)
    one8_d = nc.declare_dram_parameter("one8", [128, 32], F8, isOutput=False)
    yt_d = nc.declare_dram_parameter("yt", [L, C], BF16, isOutput=True)

    scale = float(1.0 / np.sqrt(C))

    with tile.TileContext(nc) as tc, ExitStack() as ctx:
        consts = ctx.enter_context(tc.tile_pool(name="consts", bufs=1))
        small = ctx.enter_context(tc.tile_pool(name="small", bufs=4))
        x_p = ctx.enter_context(tc.tile_pool(name="xp", bufs=4))
        scr_p = ctx.enter_context(tc.tile_pool(name="scr", bufs=2))
        h8_p = ctx.enter_context(tc.tile_pool(name="h8", bufs=2))
        q8_p = ctx.enter_context(tc.tile_pool(name="q8", bufs=2))
        k8_p = ctx.enter_context(tc.tile_pool(name="k8", bufs=2))
        v8_p = ctx.enter_context(tc.tile_pool(name="v8", bufs=2))
        w8_p = ctx.enter_context(tc.tile_pool(name="w8", bufs=4))
        wt8_p = ctx.enter_context(tc.tile_pool(name="wt8", bufs=8))
        pt_p = ctx.enter_context(tc.tile_pool(name="pt", bufs=4))
        xt_p = ctx.enter_context(tc.tile_pool(name="xtp", bufs=5))
        io_p = ctx.enter_context(tc.tile_pool(name="io", bufs=4))
        ps_mm = ctx.enter_context(tc.tile_pool(name="psmm", bufs=5, space="PSUM"))
        ps_s = ctx.enter_context(tc.tile_pool(name="pss", bufs=3, space="PSUM"))

        gmil_sb = consts.tile([128, G * CT], F32, name=nm("gmil"), tag="gmil")
        nc.sync.dma_start(out=gmil_sb[:], in_=gmil_d[:, :])
        # x tiles next on the queue: stats (and everything after) gate on the
        # full x arrival, so it precedes the other loads
        x_t = []
        for ct in range(CT):
            xin = x_p.tile([128, L], BF16, name=nm("x"), tag="x")
            x_t.append(xin)
            nc.sync.dma_start(out=xin[:], in_=x_d[ct * 128:(ct + 1) * 128, :])
        ones8 = consts.tile([128, 2, 16], F8, name=nm("ones8"), tag="ones8")
        for ko in range(2):
            nc.sync.dma_start(out=ones8[:, ko, :], in_=one8_d[:, ko * 16:(ko + 1) * 16])
        cv_sb = consts.tile([128, 5 * CT], F32, name=nm("cv"), tag="cv")
        nc.sync.dma_start(out=cv_sb[:], in_=cvec_d[:, :])
        gmT_sb = consts.tile([G, C], F32, name=nm("gmT"), tag="gmT")
        nc.sync.dma_start(out=gmT_sb[:], in_=gmT_d[:, :])

        onesf = consts.tile([1, 1], F32, name=nm("onesf"), tag="onesf")
        nc.vector.memset(onesf[:], 1.0)
        eps_t = consts.tile([G, 1], F32, name=nm("eps"), tag="eps")
        nc.vector.memset(eps_t[:], EPS)
        ebias_t = consts.tile([128, 1], F32, name=nm("ebias"), tag="ebias")
        nc.vector.memset(ebias_t[:], EXP_BIAS)

        # warm-up matmuls on the early-arriving mask tile (f32): keep the PE
        # at full clock and busy while x streams in and stats run
        for i in range(NWARM):
            wps = ps_mm.tile([128, 128], F32, name=nm("warm"), tag="mm")
            nc.tensor.matmul(wps[:], gmil_sb[:, 0:128], gmil_sb[:, 0:128],
                             start=True, stop=True)

        # fp8 weights in pair layout, 2 tiles [128, 2, C] per weight
        def load_w(w_dram):
            wsb = []
            for t in range(T):
                w = w8_p.tile([128, 2, C], F8, name=nm("w"), tag="w")
                for ko in range(2):
                    off = t * 2 * C + ko * C
                    nc.sync.dma_start(out=w[:, ko, :], in_=w_dram[:, off:off + C])
                wsb.append(w)
            return wsb

        gm_sb = [gmil_sb[:, ct * G:(ct + 1) * G] for ct in range(CT)]
        bq_t = [cv_sb[:, ct * 5 + 0:ct * 5 + 1] for ct in range(CT)]
        bk_t = [cv_sb[:, ct * 5 + 1:ct * 5 + 2] for ct in range(CT)]
        bv_t = [cv_sb[:, ct * 5 + 2:ct * 5 + 3] for ct in range(CT)]
        gnw_t = [cv_sb[:, ct * 5 + 3:ct * 5 + 4] for ct in range(CT)]
        gnb_t = [cv_sb[:, ct * 5 + 4:ct * 5 + 5] for ct in range(CT)]

        # ---- GroupNorm pass 1: per-channel sum and sum-of-squares ----
        stats = []
        for ct in range(CT):
            st = small.tile([128, 2], F32, name=nm("st"), tag=f"st{ct}")
            stats.append(st)
            xin = x_t[ct]
            nc.vector.reduce_sum(out=st[:, 0:1], in_=xin[:], axis=AX.X)
            scr = scr_p.tile([128, L], BF16, name=nm("scr"), tag="scr")
            nc.scalar.activation(out=scr[:], in_=xin[:], func=AF.Square,
                                 accum_out=st[:, 1:2])

        # group-reduce the per-channel stats: [32, 2] = sum over channels in group
        gps = ps_s.tile([G, 2], F32, name=nm("s"), tag="s")
        for ct in range(CT):
            nc.tensor.matmul(gps[:], gm_sb[ct], stats[ct][:],
                             start=(ct == 0), stop=(ct == CT - 1))
        gmv = small.tile([G, 2], F32, name=nm("gmv"), tag="gmv")
        nc.scalar.mul(out=gmv[:], in_=gps[:], mul=1.0 / (GS * L))
        msq = small.tile([G, 1], F32, name=nm("msq"), tag="msq")
        nc.vector.tensor_mul(out=msq[:], in0=gmv[:, 0:1], in1=gmv[:, 0:1])
        var = small.tile([G, 1], F32, name=nm("var"), tag="var")
        nc.vector.tensor_sub(out=var[:], in0=gmv[:, 1:2], in1=msq[:])
        rstd = small.tile([G, 1], F32, name=nm("rstd"), tag="rstd")
        nc.scalar.activation(out=rstd[:], in_=var[:], func=AF.Sqrt,
                             bias=eps_t[:], scale=1.0)
        mr = small.tile([G, 2], F32, name=nm("mr"), tag="mr")
        nc.vector.tensor_copy(out=mr[:, 0:1], in_=gmv[:, 0:1])
        nc.vector.reciprocal(out=mr[:, 1:2], in_=rstd[:])

        # broadcast group mean/rstd back to channels, fold in gn weight/bias
        s_t, t_t = [], []
        for ct in range(CT):
            bps = ps_s.tile([128, 2], F32, name=nm("s"), tag="s")
            nc.tensor.matmul(bps[:], gmT_sb[:, ct * 128:(ct + 1) * 128], mr[:],
                             start=True, stop=True)
            s_ = small.tile([128, 1], F32, name=nm("sc"), tag=f"sc{ct}")
            nc.vector.tensor_mul(out=s_[:], in0=bps[:, 1:2], in1=gnw_t[ct])
            tmp = small.tile([128, 1], F32, name=nm("tmp"), tag="tmp")
            nc.vector.tensor_mul(out=tmp[:], in0=bps[:, 0:1], in1=s_[:])
            t_ = small.tile([128, 1], F32, name=nm("tc"), tag=f"tc{ct}")
            nc.vector.tensor_sub(out=t_[:], in0=gnb_t[ct], in1=tmp[:])
            s_t.append(s_)
            t_t.append(t_)

        wq8_sb = load_w(wq8_d)

        # ---- GroupNorm pass 2: H8 = fp8(s*x + t) in pair layout,
        # chunked in conv consumption order and split across DVE/ACT ----
        h8_t = [h8_p.tile([128, 2, L], F8, name=nm("h8"), tag="h8")
                for _ in range(T)]
        for lc in range(2):
            sl = slice(lc * 1024, (lc + 1) * 1024)
            for ct in range(CT):
                dst = h8_t[ct // 2][:, ct % 2, sl]
                if ct % 2 == 0:
                    nc.vector.tensor_scalar(out=dst, in0=x_t[ct][:, sl],
                                            scalar1=s_t[ct][:],
                                            scalar2=t_t[ct][:],
                                            op0=AluOpType.mult,
                                            op1=AluOpType.add)
                else:
                    nc.scalar.activation(out=dst, in_=x_t[ct][:, sl],
                                         func=AF.Identity, bias=t_t[ct][:],
                                         scale=s_t[ct][:])

        # ---- 1x1 convs in fp8 DoubleRow; outputs in fp8 pair layout ----
        # evacs alternate DVE/ACT by lc parity so neither engine's evac
        # back-pressure stalls the PE (4 evacs/co on one engine > 8 MMs)
        def conv(bias_t, pool, tag, wsb, act_first=False):
            out8 = [pool.tile([128, 2, L], F8, name=nm(tag), tag=tag)
                    for _ in range(T)]
            for co in range(CT):
                pss = [ps_mm.tile([128, NB], F32, name=nm("mm"), tag="mm")
                       for _ in range(L // NB)]
                for lc in range(L // NB):
                    for t in range(T):
                        nc.tensor.matmul(
                            pss[lc][:],
                            wsb[t][:, :, co * 128:(co + 1) * 128],
                            h8_t[t][:, :, lc * NB:(lc + 1) * NB],
                            start=(t == 0), stop=(t == T - 1),
                            perf_mode=DR)
                for lc in range(L // NB):
                    dst = out8[co // 2][:, co % 2, lc * NB:(lc + 1) * NB]
                    if (lc % 2 == 0) == act_first:
                        nc.scalar.activation(out=dst, in_=pss[lc][:],
                                             func=AF.Identity,
                                             bias=bias_t[co], scale=1.0)
                    else:
                        nc.vector.tensor_scalar_add(
                            out=dst, in0=pss[lc][:], scalar1=bias_t[co])
            return out8

        q8_t = conv(bq_t, q8_p, "q", wq8_sb)
        wk8_sb = load_w(wk8_d)
        k8_t = conv(bk_t, k8_p, "k", wk8_sb, act_first=True)
        wv8_sb = load_w(wv8_d)
        v8_t = conv(bv_t, v8_p, "v", wv8_sb)
        wo8_sb = load_w(wo8_d)

        # ---- W~^T = (Wo V)^T, fp8 pair tiles over j ----
        wt8 = [wt8_p.tile([128, 2, C], F8, name=nm("wt"), tag="wt")
               for _ in range(JP)]
        for jt in range(JT):
            ps = ps_mm.tile([128, C], F32, name=nm("mm"), tag="mm")
            for t in range(T):
                nc.tensor.matmul(
                    ps[:],
                    v8_t[t][:, :, jt * 128:(jt + 1) * 128],
                    wo8_sb[t][:],
                    start=(t == 0), stop=(t == T - 1),
                    perf_mode=DR)
            dst = wt8[jt // 2][:, jt % 2, :]
            if jt % 2 == 0:
                nc.scalar.copy(out=dst, in_=ps[:])
            else:
                nc.vector.tensor_copy(out=dst, in_=ps[:])

        # ---- attention: blocks of 512 i columns ----
        for ib in range(LB):
            rsps = ps_s.tile([1, NB], F32, name=nm("rs"), tag="s")
            ops = [ps_mm.tile([128, C], F32, name=nm("mm"), tag="mm")
                   for _ in range(4)]
            xt_sbs = []
            for s in range(4):
                row = ib * NB + s * 128
                xt_sb = xt_p.tile([128, C], BF16, name=nm("xt"), tag="xt")
                nc.sync.dma_start(out=xt_sb[:], in_=xt_d[row:row + 128, :])
                xt_sbs.append(xt_sb)
            pts = [None] * JP

            def do_S(jt):
                jp, jo = jt // 2, jt % 2
                if jo == 0:
                    pts[jp] = pt_p.tile([128, 2, NB], F8, name=nm("p"), tag="p")
                sps = ps_s.tile([128, NB], F32, name=nm("s"), tag="s")
                for t in range(T):
                    nc.tensor.matmul(
                        sps[:],
                        k8_t[t][:, :, jt * 128:(jt + 1) * 128],
                        q8_t[t][:, :, ib * NB:(ib + 1) * NB],
                        start=(t == 0), stop=(t == T - 1),
                        perf_mode=DR)
                nc.scalar.activation(out=pts[jp][:, jo, :], in_=sps[:],
                                     func=AF.Exp, scale=scale, bias=ebias_t[:])

            def do_out(jp):
                # row sums first: the block-tail normalize chain hangs off
                # this, so it should finish before the last out matmuls
                nc.tensor.matmul(rsps[:], ones8[:, :, 0:1], pts[jp][:],
                                 start=(jp == 0), stop=(jp == JP - 1),
                                 perf_mode=DR)
                for s in range(4):
                    nc.tensor.matmul(ops[s][:],
                                     pts[jp][:, :, s * 128:(s + 1) * 128],
                                     wt8[jp][:],
                                     start=(jp == 0), stop=(jp == JP - 1),
                                     perf_mode=DR)

            # software pipeline: S/exp of pair jp+1 is emitted before the
            # out matmuls of pair jp, so the PE streams S matmuls while the
            # ACT exp for the previous pair is still in flight
            for jp in range(JP):
                do_S(2 * jp)
                do_S(2 * jp + 1)
                if jp > 0:
                    do_out(jp - 1)
            do_out(JP - 1)
            rssb = small.tile([1, NB], F32, name=nm("rssb"), tag="rssb")
            nc.vector.tensor_copy(out=rssb[:], in_=rsps[:])
            rec4 = small.tile([128, 4], F32, name=nm("rec4"), tag="rec4")
            if ib < LB - 1:
                # mid-block: DMA scatter (PE is busy with the next block)
                rs4 = small.tile([128, 4], F32, name=nm("rs4"), tag="rs4")
                for s in range(4):
                    nc.sync.dma_start(out=rs4[:, s:s + 1],
                                      in_=rssb[0:1, s * 128:(s + 1) * 128])
                nc.vector.reciprocal(out=rec4[:], in_=rs4[:])
            else:
                # last block: K=1 transpose matmuls (PE idle, shortest chain)
                trp = ps_s.tile([128, 4], F32, name=nm("tr"), tag="s")
                for s in range(4):
                    nc.tensor.matmul(trp[:, s:s + 1],
                                     rssb[0:1, s * 128:(s + 1) * 128],
                                     onesf[:],
                                     start=True, stop=True)
                nc.vector.reciprocal(out=rec4[:], in_=trp[:])
            for s in range(4):
                row = ib * NB + s * 128
                yt_sb = io_p.tile([128, C], BF16, name=nm("yt"), tag="yt")
                if ib == LB - 1 and s % 2 == 1:
                    # last block: odd slices go ACT(scale) + GpSimd(add) so
                    # the tail drains on three engines instead of one
                    o1 = io_p.tile([128, C], F32, name=nm("o1"), tag="o1")
                    nc.scalar.activation(out=o1[:], in_=ops[s][:],
                                         func=AF.Copy,
                                         scale=rec4[:, s:s + 1])
                    nc.gpsimd.tensor_add(out=yt_sb[:], in0=o1[:],
                                         in1=xt_sbs[s][:])
                else:
                    # fused: yt = ops[s] * (1/rowsum) + (x^T + bo), bf16 out
                    nc.vector.scalar_tensor_tensor(
                        out=yt_sb[:], in0=ops[s][:], scalar=rec4[:, s:s + 1],
                        in1=xt_sbs[s][:],
                        op0=AluOpType.mult, op1=AluOpType.add)
                nc.sync.dma_start(out=yt_d[row:row + 128, :], in_=yt_sb[:])

    nc.compile()
    return nc


def get_nc():
    if "nc" not in _CACHE:
        _CACHE["nc"] = _build()
    return _CACHE["nc"]


def _pair8(wT):
    # wT: [C, O] f32, c -> (t = c//256, ko = (c//128)%2, ki = c%128)
    # returns [128, T*2*O] fp8: free index = t*(2*O) + ko*O + o
    O = wT.shape[1]
    arr = wT.reshape(T, 2, 128, O).transpose(2, 0, 1, 3).reshape(128, T * 2 * O)
    return np.ascontiguousarray(arr.astype(ml_dtypes.float8_e4m3))


def make_in_maps(**inputs):
    x = np.asarray(inputs["x"], np.float32)
    bo = np.asarray(inputs["bo"], np.float32)
    gm = np.zeros((C, G), np.float32)
    gm[np.arange(C), np.arange(C) // GS] = 1.0
    shared = {
        "wq8": _pair8(np.asarray(inputs["wq"], np.float32).T),
        "wk8": _pair8(np.asarray(inputs["wk"], np.float32).T),
        "wv8": _pair8(np.asarray(inputs["wv"], np.float32).T),
        "wo8": _pair8(np.asarray(inputs["wo"], np.float32).T),
        "cvec": np.stack(
            [np.asarray(inputs[k], np.float32).reshape(CT, 128)
             for k in ("bq", "bk", "bv", "gn_w", "gn_b")],
            axis=-1).transpose(1, 0, 2).reshape(128, CT * 5).copy(),
        "gmil": gm.reshape(CT, 128, G).transpose(1, 0, 2).reshape(128, CT * G).copy(),
        "gmT": np.ascontiguousarray(gm.T),
        "one8": np.ones((128, 32), ml_dtypes.float8_e4m3),
    }
    in_maps = []
    for b in range(NCORES):
        m = dict(shared)
        m["x"] = np.ascontiguousarray(x[b]).astype(ml_dtypes.bfloat16)
        m["xt"] = np.ascontiguousarray(x[b].T + bo[None, :]).astype(ml_dtypes.bfloat16)
        in_maps.append(m)
    return in_maps


def kernel(**inputs):
    from concourse.bass_utils import run_bass_kernel_spmd

    nc = get_nc()
    in_maps = make_in_maps(**inputs)
    res = run_bass_kernel_spmd(nc, in_maps, core_ids=list(range(NCORES)))
    out = np.stack([np.asarray(res.results[b]["yt"]).astype(np.float32).T
                    for b in range(NCORES)])
    return np.ascontiguousarray(out, dtype=np.float32)
